# revision 4
# baseline (speedup 1.0000x reference)
"""Stereo cost-volume builder (nn_CostBuilder) as a Trainium2 Bass kernel.

Reference op: out[b, 0:C,  d, h, w] = left[b, c, h, w]   * (w >= d)
              out[b, C:2C, d, h, w] = right[b, c, h, w-d] * (w >= d)
with B=4, C=32, D=48, H=64, W=128 (f32). Output is [4, 64, 48, 64, 128].

Sharding across 8 cores: core m -> (b = m//2, d-half = m%2). Each core
produces out[b, :, d0:d0+24, :, :] (50.3 MB), i.e. both the left-masked and
right-shifted channels for 24 of the 48 disparities. The program is uniform
(true SPMD): the disparity offset d0 only changes per-core *data* (the mask
tensor and the host-side shift baked into the padded right features).

The op is write-bandwidth-bound (output is 48x the input), so the layout is
chosen to make output DMA descriptors fat: SBUF partition = (channel,
h-quarter) so each partition holds 16 h-rows, making every descriptor an
8 KB contiguous run on both the SBUF and DRAM side (v1 used 512 B runs and
was descriptor-rate-limited at ~197 GB/s/core).

Per d-chunk of 3 disparities (8 chunks):
  - left:  one DVE tensor_mul  [128, 3*16*128] = row * mask(d, w)
  - right: one ACT shifted copy [128, 3*16*128] from the zero-padded right
           rows (src AP steps: d=-1, h=+176, w=+1), realizing shift-by-d
           with zero fill.
  - 32+32 per-channel DMAs out (96 KB each, 12 descriptors of 8 KB).
"""

import sys

if "/opt/trn_rl_repo" not in sys.path:
    sys.path.insert(0, "/opt/trn_rl_repo")

import numpy as np

import concourse.bacc as bacc
import concourse.bass as bass
import concourse.mybir as mybir
import concourse.tile as tile
from concourse.bass_utils import run_bass_kernel_spmd

B, C, H, W = 4, 32, 64, 128
D = 48          # MAX_DISP // 4
DD = D // 2     # disparities per core
N_CORES = 8
PAD = DD + DD + W  # 176 cols per padded right row
HP = 16         # h-rows per partition; partition = (c, h//HP), 32*4 = 128
NHQ = H // HP   # 4 h-quarters
DC = 3          # disparities per chunk
NCHUNK = DD // DC  # 8 chunks
FB = HP * W     # 2048: elements per (c, d, h-quarter) block = one 8KB descriptor

_NC_CACHE = {}


def _build_nc():
    nc = bacc.Bacc("TRN2", target_bir_lowering=False, debug=False)
    f32 = mybir.dt.float32

    lfeat = nc.dram_tensor("lfeat", [C, H, W], f32, kind="ExternalInput").ap()
    rpad = nc.dram_tensor("rpad", [C, H, PAD], f32, kind="ExternalInput").ap()
    lmask = nc.dram_tensor("lmask", [128, DD * W], f32, kind="ExternalInput").ap()
    out = nc.dram_tensor("out", [2 * C, DD, H, W], f32, kind="ExternalOutput").ap()

    c_str = DD * H * W  # 196608: channel stride in `out`

    with tile.TileContext(nc) as tc:
        with (
            tc.tile_pool(name="consts", bufs=1) as const_pool,
            tc.tile_pool(name="lst", bufs=3) as lst_pool,
            tc.tile_pool(name="rst", bufs=3) as rst_pool,
        ):
            # whole-problem inputs, loaded once
            mtile = const_pool.tile([128, DD * W], f32, name="mtile")
            nc.gpsimd.dma_start(mtile[:], lmask)
            ltile = const_pool.tile([128, HP * W], f32, name="ltile")
            nc.gpsimd.dma_start(ltile[:], lfeat[:])
            rtile = const_pool.tile([128, HP * PAD], f32, name="rtile")
            nc.gpsimd.dma_start(rtile[:], rpad[:])

            for k in range(NCHUNK):
                d0k = DC * k
                lstage = lst_pool.tile([128, DC * FB], f32, name="lstage")
                rstage = rst_pool.tile([128, DC * FB], f32, name="rstage")

                # left: lstage[p, d', hh, w] = ltile[p, hh, w] * mask[d0k+d', w]
                nc.vector.tensor_mul(
                    lstage[:].rearrange("p (d hh w) -> p d hh w", d=DC, hh=HP),
                    ltile[:]
                    .rearrange("p (hh w) -> p hh w", hh=HP)
                    .unsqueeze(1)
                    .to_broadcast((128, DC, HP, W)),
                    mtile[:, W * d0k : W * (d0k + DC)]
                    .rearrange("p (d w) -> p d w", d=DC)
                    .unsqueeze(2)
                    .to_broadcast((128, DC, HP, W)),
                )

                # right: rstage[p, d', hh, w] = rtile[p, hh, DD + w - (d0k+d')]
                sR = rtile[:, (DD - d0k) : (DD - d0k) + 1]
                srcR = bass.AP(
                    sR.tensor,
                    sR.offset,
                    [[HP * PAD, 128], [-1, DC], [PAD, HP], [1, W]],
                )
                nc.scalar.copy(
                    rstage[:].rearrange("p (d hh w) -> p d hh w", d=DC, hh=HP),
                    srcR,
                )

                # per-channel DMAs out: src [hq(4), d'(3), 2048] (3-dim limit),
                # every (hq, d') pair is one 8KB contiguous run on both sides.
                sL, sRs = lstage[:], rstage[:]
                for c in range(C):
                    srcLc = bass.AP(
                        sL.tensor,
                        sL.offset + c * NHQ * (DC * FB),
                        [[DC * FB, NHQ], [FB, DC], [1, FB]],
                    )
                    dstLc = bass.AP(
                        out.tensor,
                        c * c_str + d0k * H * W,
                        [[FB, NHQ], [H * W, DC], [1, FB]],
                    )
                    nc.sync.dma_start(dstLc, srcLc)

                    srcRc = bass.AP(
                        sRs.tensor,
                        sRs.offset + c * NHQ * (DC * FB),
                        [[DC * FB, NHQ], [FB, DC], [1, FB]],
                    )
                    dstRc = bass.AP(
                        out.tensor,
                        (C + c) * c_str + d0k * H * W,
                        [[FB, NHQ], [H * W, DC], [1, FB]],
                    )
                    nc.scalar.dma_start(dstRc, srcRc)

    nc.compile()
    return nc


def get_nc():
    if "nc" not in _NC_CACHE:
        _NC_CACHE["nc"] = _build_nc()
    return _NC_CACHE["nc"]


def make_in_maps(left, right):
    """Per-core input dicts for run_bass_kernel_spmd."""
    left = np.ascontiguousarray(left, dtype=np.float32)
    right = np.ascontiguousarray(right, dtype=np.float32)
    ds = np.arange(DD)[:, None]
    w = np.arange(W)[None, :]
    in_maps = []
    for m in range(N_CORES):
        b, dh = divmod(m, 2)
        d0 = DD * dh
        rpad = np.zeros((C, H, PAD), np.float32)
        rpad[:, :, DD + d0 : DD + d0 + W] = right[b]
        mrow = (w >= (d0 + ds)).astype(np.float32).reshape(1, DD * W)
        lmask = np.ascontiguousarray(np.broadcast_to(mrow, (128, DD * W)))
        in_maps.append(
            {"lfeat": np.ascontiguousarray(left[b]), "rpad": rpad, "lmask": lmask}
        )
    return in_maps


def assemble(results):
    """Gather per-core [2C, DD, H, W] chunks into the full [B, 2C, D, H, W]."""
    full = np.empty((B, 2 * C, D, H, W), np.float32)
    for m in range(N_CORES):
        b, dh = divmod(m, 2)
        full[b, :, DD * dh : DD * dh + DD] = results[m]["out"]
    return full


def kernel(**inputs):
    nc = get_nc()
    in_maps = make_in_maps(inputs["left_feats"], inputs["right_feats"])
    res = run_bass_kernel_spmd(nc, in_maps, list(range(N_CORES))).results
    return assemble(res)


# revision 6
# speedup vs baseline: 3.3535x; 3.3535x over previous
"""Stereo cost-volume builder (nn_CostBuilder) as a Trainium2 Bass kernel.

Reference op: out[b, 0:C,  d, h, w] = left[b, c, h, w]   * (w >= d)
              out[b, C:2C, d, h, w] = right[b, c, h, w-d] * (w >= d)
with B=4, C=32, D=48, H=64, W=128 (f32). Output is [4, 64, 48, 64, 128].

Sharding across 8 cores: core m -> (b = m//2, d-half = m%2). Each core
produces out[b, :, d0:d0+24, :, :] (50.3 MB), i.e. both the left-masked and
right-shifted channels for 24 of the 48 disparities. The program is uniform
(true SPMD): the disparity offset d0 only changes per-core *data* (the mask
tensor and the host-side shift baked into the padded right features).

The op is write-bandwidth-bound (output is 48x the input), so the layout is
chosen to make output DMA descriptors fat: SBUF partition = (channel,
h-quarter) so each partition holds 16 h-rows, making every descriptor an
8 KB contiguous run on both the SBUF and DRAM side (v1 used 512 B runs and
was descriptor-rate-limited at ~197 GB/s/core).

Per d-chunk of 3 disparities (8 chunks):
  - left:  one DVE tensor_mul  [128, 3*16*128] = row * mask(d, w)
  - right: one ACT shifted copy [128, 3*16*128] from the zero-padded right
           rows (src AP steps: d=-1, h=+176, w=+1), realizing shift-by-d
           with zero fill.
  - 32+32 per-channel DMAs out (96 KB each, 12 descriptors of 8 KB).
"""

import sys

if "/opt/trn_rl_repo" not in sys.path:
    sys.path.insert(0, "/opt/trn_rl_repo")

import numpy as np

import concourse.bacc as bacc
import concourse.bass as bass
import concourse.mybir as mybir
import concourse.tile as tile
from concourse.bass_utils import run_bass_kernel_spmd

B, C, H, W = 4, 32, 64, 128
D = 48          # MAX_DISP // 4
DD = D // 2     # disparities per core
N_CORES = 8
PAD = DD + DD + W  # 176 cols per padded right row
HP = 16         # h-rows per partition; partition = (c, h//HP), 32*4 = 128
NHQ = H // HP   # 4 h-quarters
DC = 3          # disparities per chunk
NCHUNK = DD // DC  # 8 chunks
FB = HP * W     # 2048: elements per (c, d, h-quarter) block = one 8KB descriptor

_NC_CACHE = {}


def _build_nc():
    nc = bacc.Bacc("TRN2", target_bir_lowering=False, debug=False)
    f32 = mybir.dt.float32

    lfeat = nc.dram_tensor("lfeat", [C, H, W], f32, kind="ExternalInput").ap()
    rpad = nc.dram_tensor("rpad", [C, H, PAD], f32, kind="ExternalInput").ap()
    lmask = nc.dram_tensor("lmask", [128, DD * W], f32, kind="ExternalInput").ap()
    out = nc.dram_tensor("out", [2 * C, DD, H, W], f32, kind="ExternalOutput").ap()

    c_str = DD * H * W  # 196608: channel stride in `out`

    with tile.TileContext(nc) as tc:
        with (
            tc.tile_pool(name="consts", bufs=1) as const_pool,
            tc.tile_pool(name="lst", bufs=3) as lst_pool,
            tc.tile_pool(name="rst", bufs=3) as rst_pool,
        ):
            # whole-problem inputs, loaded once
            mtile = const_pool.tile([128, DD * W], f32, name="mtile")
            nc.gpsimd.dma_start(mtile[:], lmask)
            ltile = const_pool.tile([128, HP * W], f32, name="ltile")
            nc.gpsimd.dma_start(ltile[:], lfeat[:])
            rtile = const_pool.tile([128, HP * PAD], f32, name="rtile")
            nc.gpsimd.dma_start(rtile[:], rpad[:])

            for k in range(NCHUNK):
                d0k = DC * k
                lstage = lst_pool.tile([128, DC * FB], f32, name="lstage")
                rstage = rst_pool.tile([128, DC * FB], f32, name="rstage")

                # left: lstage[p, d', hh, w] = ltile[p, hh, w] * mask[d0k+d', w]
                nc.vector.tensor_mul(
                    lstage[:].rearrange("p (d hh w) -> p d hh w", d=DC, hh=HP),
                    ltile[:]
                    .rearrange("p (hh w) -> p hh w", hh=HP)
                    .unsqueeze(1)
                    .to_broadcast((128, DC, HP, W)),
                    mtile[:, W * d0k : W * (d0k + DC)]
                    .rearrange("p (d w) -> p d w", d=DC)
                    .unsqueeze(2)
                    .to_broadcast((128, DC, HP, W)),
                )

                # right: rstage[p, d', hh, w] = rtile[p, hh, DD + w - (d0k+d')]
                sR = rtile[:, (DD - d0k) : (DD - d0k) + 1]
                srcR = bass.AP(
                    sR.tensor,
                    sR.offset,
                    [[HP * PAD, 128], [-1, DC], [PAD, HP], [1, W]],
                )
                nc.scalar.copy(
                    rstage[:].rearrange("p (d hh w) -> p d hh w", d=DC, hh=HP),
                    srcR,
                )

                # DMAs out: one 1MB DMA per (chunk, d', side) covering all 32
                # channels x 4 h-quarters = 128 partitions -> 128 descriptors
                # of 8KB each, spread across all 16 SDMA ports.
                sL, sRs = lstage[:], rstage[:]
                for dp in range(DC):
                    srcLd = bass.AP(
                        sL.tensor,
                        sL.offset + dp * FB,
                        [[DC * FB, 128], [1, FB]],
                    )
                    dstLd = bass.AP(
                        out.tensor,
                        (d0k + dp) * H * W,
                        [[c_str, C], [FB, NHQ], [1, FB]],
                    )
                    nc.sync.dma_start(dstLd, srcLd)

                    srcRd = bass.AP(
                        sRs.tensor,
                        sRs.offset + dp * FB,
                        [[DC * FB, 128], [1, FB]],
                    )
                    dstRd = bass.AP(
                        out.tensor,
                        C * c_str + (d0k + dp) * H * W,
                        [[c_str, C], [FB, NHQ], [1, FB]],
                    )
                    nc.scalar.dma_start(dstRd, srcRd)

    nc.compile()
    return nc


def get_nc():
    if "nc" not in _NC_CACHE:
        _NC_CACHE["nc"] = _build_nc()
    return _NC_CACHE["nc"]


def make_in_maps(left, right):
    """Per-core input dicts for run_bass_kernel_spmd."""
    left = np.ascontiguousarray(left, dtype=np.float32)
    right = np.ascontiguousarray(right, dtype=np.float32)
    ds = np.arange(DD)[:, None]
    w = np.arange(W)[None, :]
    in_maps = []
    for m in range(N_CORES):
        b, dh = divmod(m, 2)
        d0 = DD * dh
        rpad = np.zeros((C, H, PAD), np.float32)
        rpad[:, :, DD + d0 : DD + d0 + W] = right[b]
        mrow = (w >= (d0 + ds)).astype(np.float32).reshape(1, DD * W)
        lmask = np.ascontiguousarray(np.broadcast_to(mrow, (128, DD * W)))
        in_maps.append(
            {"lfeat": np.ascontiguousarray(left[b]), "rpad": rpad, "lmask": lmask}
        )
    return in_maps


def assemble(results):
    """Gather per-core [2C, DD, H, W] chunks into the full [B, 2C, D, H, W]."""
    full = np.empty((B, 2 * C, D, H, W), np.float32)
    for m in range(N_CORES):
        b, dh = divmod(m, 2)
        full[b, :, DD * dh : DD * dh + DD] = results[m]["out"]
    return full


def kernel(**inputs):
    nc = get_nc()
    in_maps = make_in_maps(inputs["left_feats"], inputs["right_feats"])
    res = run_bass_kernel_spmd(nc, in_maps, list(range(N_CORES))).results
    return assemble(res)


# revision 7
# speedup vs baseline: 3.8118x; 1.1366x over previous
"""Stereo cost-volume builder (nn_CostBuilder) as a Trainium2 Bass kernel.

Reference op: out[b, 0:C,  d, h, w] = left[b, c, h, w]   * (w >= d)
              out[b, C:2C, d, h, w] = right[b, c, h, w-d] * (w >= d)
with B=4, C=32, D=48, H=64, W=128 (f32). Output is [4, 64, 48, 64, 128].

Sharding across 8 cores: core m -> (b = m//2, d-half = m%2). Each core
produces out[b, :, d0:d0+24, :, :] (50.3 MB), i.e. both the left-masked and
right-shifted channels for 24 of the 48 disparities. The program is uniform
(true SPMD): the disparity offset d0 only changes per-core *data* (the mask
tensor and the host-side shift baked into the padded right features).

The op is write-bandwidth-bound (output is 48x the input), so the layout is
chosen to make output DMA descriptors fat: SBUF partition = (channel,
h-quarter) so each partition holds 16 h-rows, making every descriptor an
8 KB contiguous run on both the SBUF and DRAM side (v1 used 512 B runs and
was descriptor-rate-limited at ~197 GB/s/core).

Per d-chunk of 3 disparities (8 chunks):
  - left:  one DVE tensor_mul  [128, 3*16*128] = row * mask(d, w)
  - right: one ACT shifted copy [128, 3*16*128] from the zero-padded right
           rows (src AP steps: d=-1, h=+176, w=+1), realizing shift-by-d
           with zero fill.
  - 32+32 per-channel DMAs out (96 KB each, 12 descriptors of 8 KB).
"""

import sys

if "/opt/trn_rl_repo" not in sys.path:
    sys.path.insert(0, "/opt/trn_rl_repo")

import numpy as np

import concourse.bacc as bacc
import concourse.bass as bass
import concourse.mybir as mybir
import concourse.tile as tile
from concourse.bass_utils import run_bass_kernel_spmd

B, C, H, W = 4, 32, 64, 128
D = 48          # MAX_DISP // 4
DD = D // 2     # disparities per core
N_CORES = 8
PAD = DD + DD + W  # 176 cols per padded right row
HP = 16         # h-rows per partition; partition = (c, h//HP), 32*4 = 128
NHQ = H // HP   # 4 h-quarters
DC = 3          # disparities per chunk
NCHUNK = DD // DC  # 8 chunks
FB = HP * W     # 2048: elements per (c, d, h-quarter) block = one 8KB descriptor

_NC_CACHE = {}


def _build_nc():
    nc = bacc.Bacc("TRN2", target_bir_lowering=False, debug=False)
    f32 = mybir.dt.float32

    lfeat = nc.dram_tensor("lfeat", [C, H, W], f32, kind="ExternalInput").ap()
    rpad = nc.dram_tensor("rpad", [C, H, PAD], f32, kind="ExternalInput").ap()
    lmask = nc.dram_tensor("lmask", [128, DD * W], f32, kind="ExternalInput").ap()
    out = nc.dram_tensor("out", [2 * C, DD, H, W], f32, kind="ExternalOutput").ap()

    c_str = DD * H * W  # 196608: channel stride in `out`

    with tile.TileContext(nc) as tc:
        with (
            tc.tile_pool(name="consts", bufs=1) as const_pool,
            tc.tile_pool(name="lst", bufs=3) as lst_pool,
            tc.tile_pool(name="rst", bufs=3) as rst_pool,
        ):
            # whole-problem inputs, loaded once; one load per DMA path so they
            # run in parallel (sync/scalar = the two HWDGE rings, gpsimd = SWDGE)
            ltile = const_pool.tile([128, HP * W], f32, name="ltile")
            nc.sync.dma_start(ltile[:], lfeat[:])
            rtile = const_pool.tile([128, HP * PAD], f32, name="rtile")
            nc.scalar.dma_start(rtile[:], rpad[:])
            mtile = const_pool.tile([128, DD * W], f32, name="mtile")
            nc.gpsimd.dma_start(mtile[:], lmask)

            for k in range(NCHUNK):
                d0k = DC * k
                lstage = lst_pool.tile([128, DC * FB], f32, name="lstage")
                rstage = rst_pool.tile([128, DC * FB], f32, name="rstage")

                # left: lstage[p, d', hh, w] = ltile[p, hh, w] * mask[d0k+d', w]
                nc.vector.tensor_mul(
                    lstage[:].rearrange("p (d hh w) -> p d hh w", d=DC, hh=HP),
                    ltile[:]
                    .rearrange("p (hh w) -> p hh w", hh=HP)
                    .unsqueeze(1)
                    .to_broadcast((128, DC, HP, W)),
                    mtile[:, W * d0k : W * (d0k + DC)]
                    .rearrange("p (d w) -> p d w", d=DC)
                    .unsqueeze(2)
                    .to_broadcast((128, DC, HP, W)),
                )

                # right: rstage[p, d', hh, w] = rtile[p, hh, DD + w - (d0k+d')]
                sR = rtile[:, (DD - d0k) : (DD - d0k) + 1]
                srcR = bass.AP(
                    sR.tensor,
                    sR.offset,
                    [[HP * PAD, 128], [-1, DC], [PAD, HP], [1, W]],
                )
                nc.scalar.copy(
                    rstage[:].rearrange("p (d hh w) -> p d hh w", d=DC, hh=HP),
                    srcR,
                )

                # DMAs out: one 1MB DMA per (chunk, d', side) covering all 32
                # channels x 4 h-quarters = 128 partitions -> 128 descriptors
                # of 8KB each, spread across all 16 SDMA ports.
                sL, sRs = lstage[:], rstage[:]
                for dp in range(DC):
                    srcLd = bass.AP(
                        sL.tensor,
                        sL.offset + dp * FB,
                        [[DC * FB, 128], [1, FB]],
                    )
                    dstLd = bass.AP(
                        out.tensor,
                        (d0k + dp) * H * W,
                        [[c_str, C], [FB, NHQ], [1, FB]],
                    )
                    nc.sync.dma_start(dstLd, srcLd)

                    srcRd = bass.AP(
                        sRs.tensor,
                        sRs.offset + dp * FB,
                        [[DC * FB, 128], [1, FB]],
                    )
                    dstRd = bass.AP(
                        out.tensor,
                        C * c_str + (d0k + dp) * H * W,
                        [[c_str, C], [FB, NHQ], [1, FB]],
                    )
                    nc.scalar.dma_start(dstRd, srcRd)

    nc.compile()
    return nc


def get_nc():
    if "nc" not in _NC_CACHE:
        _NC_CACHE["nc"] = _build_nc()
    return _NC_CACHE["nc"]


def make_in_maps(left, right):
    """Per-core input dicts for run_bass_kernel_spmd."""
    left = np.ascontiguousarray(left, dtype=np.float32)
    right = np.ascontiguousarray(right, dtype=np.float32)
    ds = np.arange(DD)[:, None]
    w = np.arange(W)[None, :]
    in_maps = []
    for m in range(N_CORES):
        b, dh = divmod(m, 2)
        d0 = DD * dh
        rpad = np.zeros((C, H, PAD), np.float32)
        rpad[:, :, DD + d0 : DD + d0 + W] = right[b]
        mrow = (w >= (d0 + ds)).astype(np.float32).reshape(1, DD * W)
        lmask = np.ascontiguousarray(np.broadcast_to(mrow, (128, DD * W)))
        in_maps.append(
            {"lfeat": np.ascontiguousarray(left[b]), "rpad": rpad, "lmask": lmask}
        )
    return in_maps


def assemble(results):
    """Gather per-core [2C, DD, H, W] chunks into the full [B, 2C, D, H, W]."""
    full = np.empty((B, 2 * C, D, H, W), np.float32)
    for m in range(N_CORES):
        b, dh = divmod(m, 2)
        full[b, :, DD * dh : DD * dh + DD] = results[m]["out"]
    return full


def kernel(**inputs):
    nc = get_nc()
    in_maps = make_in_maps(inputs["left_feats"], inputs["right_feats"])
    res = run_bass_kernel_spmd(nc, in_maps, list(range(N_CORES))).results
    return assemble(res)


# revision 9
# speedup vs baseline: 3.9083x; 1.0253x over previous
"""Stereo cost-volume builder (nn_CostBuilder) as a Trainium2 Bass kernel.

Reference op: out[b, 0:C,  d, h, w] = left[b, c, h, w]   * (w >= d)
              out[b, C:2C, d, h, w] = right[b, c, h, w-d] * (w >= d)
with B=4, C=32, D=48, H=64, W=128 (f32). Output is [4, 64, 48, 64, 128].

Sharding across 8 cores: core m -> (b = m//2, d-half = m%2). Each core
produces out[b, :, d0:d0+24, :, :] (50.3 MB), i.e. both the left-masked and
right-shifted channels for 24 of the 48 disparities. The program is uniform
(true SPMD): the disparity offset d0 only changes per-core *data* (the mask
tensor and the host-side shift baked into the padded right features).

The op is write-bandwidth-bound (output is 48x the input), so the layout is
chosen to make output DMA descriptors fat: SBUF partition = (channel,
h-quarter) so each partition holds 16 h-rows, making every descriptor an
8 KB contiguous run on both the SBUF and DRAM side (v1 used 512 B runs and
was descriptor-rate-limited at ~197 GB/s/core).

Per d-chunk of 3 disparities (8 chunks):
  - left:  one DVE tensor_mul  [128, 3*16*128] = row * mask(d, w)
  - right: one ACT shifted copy [128, 3*16*128] from the zero-padded right
           rows (src AP steps: d=-1, h=+176, w=+1), realizing shift-by-d
           with zero fill.
  - 32+32 per-channel DMAs out (96 KB each, 12 descriptors of 8 KB).
"""

import sys

if "/opt/trn_rl_repo" not in sys.path:
    sys.path.insert(0, "/opt/trn_rl_repo")

import numpy as np

import concourse.bacc as bacc
import concourse.bass as bass
import concourse.mybir as mybir
import concourse.tile as tile
from concourse.bass_utils import run_bass_kernel_spmd

B, C, H, W = 4, 32, 64, 128
D = 48          # MAX_DISP // 4
DD = D // 2     # disparities per core
N_CORES = 8
PAD = DD + DD + W  # 176 cols per padded right row
HP = 16         # h-rows per partition; partition = (c, h//HP), 32*4 = 128
NHQ = H // HP   # 4 h-quarters
DC = 3          # disparities per chunk
NCHUNK = DD // DC  # 8 chunks
FB = HP * W     # 2048: elements per (c, d, h-quarter) block = one 8KB descriptor

_NC_CACHE = {}


def _build_nc():
    nc = bacc.Bacc("TRN2", target_bir_lowering=False, debug=False)
    f32 = mybir.dt.float32

    lfeat = nc.dram_tensor("lfeat", [C, H, W], f32, kind="ExternalInput").ap()
    rpad = nc.dram_tensor("rpad", [C, H, PAD], f32, kind="ExternalInput").ap()
    dzero = nc.dram_tensor("dzero", [128, 1], f32, kind="ExternalInput").ap()
    out = nc.dram_tensor("out", [2 * C, DD, H, W], f32, kind="ExternalOutput").ap()

    c_str = DD * H * W  # 196608: channel stride in `out`

    with tile.TileContext(nc) as tc:
        with (
            tc.tile_pool(name="consts", bufs=1) as const_pool,
            tc.tile_pool(name="lst", bufs=3) as lst_pool,
            tc.tile_pool(name="rst", bufs=3) as rst_pool,
        ):
            # whole-problem inputs, loaded once; one load per DMA path so they
            # run in parallel (sync/scalar = the two HWDGE rings, gpsimd = SWDGE)
            ltile = const_pool.tile([128, HP * W], f32, name="ltile")
            nc.sync.dma_start(ltile[:], lfeat[:])
            rtile = const_pool.tile([128, HP * PAD], f32, name="rtile")
            nc.scalar.dma_start(rtile[:], rpad[:])
            # mask built on-device: mask[p, d*W+w] = (w - d >= d0), d0 is the
            # per-core disparity offset fed as a tiny [128,1] int32 input
            dztile = const_pool.tile([128, 1], f32, name="dztile")
            nc.gpsimd.dma_start(dztile[:], dzero)
            itile = const_pool.tile([128, DD * W], f32, name="itile")
            nc.gpsimd.iota(
                itile[:],
                [[-1, DD], [1, W]],
                channel_multiplier=0,
                allow_small_or_imprecise_dtypes=True,
            )
            mtile = const_pool.tile([128, DD * W], f32, name="mtile")
            nc.vector.tensor_scalar(
                out=mtile[:],
                in0=itile[:],
                scalar1=dztile[:],
                scalar2=None,
                op0=mybir.AluOpType.is_ge,
            )

            for k in range(NCHUNK):
                d0k = DC * k
                lstage = lst_pool.tile([128, DC * FB], f32, name="lstage")
                rstage = rst_pool.tile([128, DC * FB], f32, name="rstage")

                # left: lstage[p, d', hh, w] = ltile[p, hh, w] * mask[d0k+d', w]
                nc.vector.tensor_mul(
                    lstage[:].rearrange("p (d hh w) -> p d hh w", d=DC, hh=HP),
                    ltile[:]
                    .rearrange("p (hh w) -> p hh w", hh=HP)
                    .unsqueeze(1)
                    .to_broadcast((128, DC, HP, W)),
                    mtile[:, W * d0k : W * (d0k + DC)]
                    .rearrange("p (d w) -> p d w", d=DC)
                    .unsqueeze(2)
                    .to_broadcast((128, DC, HP, W)),
                )

                # right: rstage[p, d', hh, w] = rtile[p, hh, DD + w - (d0k+d')]
                sR = rtile[:, (DD - d0k) : (DD - d0k) + 1]
                srcR = bass.AP(
                    sR.tensor,
                    sR.offset,
                    [[HP * PAD, 128], [-1, DC], [PAD, HP], [1, W]],
                )
                nc.scalar.copy(
                    rstage[:].rearrange("p (d hh w) -> p d hh w", d=DC, hh=HP),
                    srcR,
                )

                # DMAs out: one 1MB DMA per (chunk, d', side) covering all 32
                # channels x 4 h-quarters = 128 partitions -> 128 descriptors
                # of 8KB each, spread across all 16 SDMA ports.
                sL, sRs = lstage[:], rstage[:]
                for dp in range(DC):
                    srcLd = bass.AP(
                        sL.tensor,
                        sL.offset + dp * FB,
                        [[DC * FB, 128], [1, FB]],
                    )
                    dstLd = bass.AP(
                        out.tensor,
                        (d0k + dp) * H * W,
                        [[c_str, C], [FB, NHQ], [1, FB]],
                    )
                    nc.sync.dma_start(dstLd, srcLd)

                    srcRd = bass.AP(
                        sRs.tensor,
                        sRs.offset + dp * FB,
                        [[DC * FB, 128], [1, FB]],
                    )
                    dstRd = bass.AP(
                        out.tensor,
                        C * c_str + (d0k + dp) * H * W,
                        [[c_str, C], [FB, NHQ], [1, FB]],
                    )
                    nc.scalar.dma_start(dstRd, srcRd)

    nc.compile()
    return nc


def get_nc():
    if "nc" not in _NC_CACHE:
        _NC_CACHE["nc"] = _build_nc()
    return _NC_CACHE["nc"]


def make_in_maps(left, right):
    """Per-core input dicts for run_bass_kernel_spmd."""
    left = np.ascontiguousarray(left, dtype=np.float32)
    right = np.ascontiguousarray(right, dtype=np.float32)
    in_maps = []
    for m in range(N_CORES):
        b, dh = divmod(m, 2)
        d0 = DD * dh
        rpad = np.zeros((C, H, PAD), np.float32)
        rpad[:, :, DD + d0 : DD + d0 + W] = right[b]
        dzero = np.full((128, 1), d0, np.float32)
        in_maps.append(
            {"lfeat": np.ascontiguousarray(left[b]), "rpad": rpad, "dzero": dzero}
        )
    return in_maps


def assemble(results):
    """Gather per-core [2C, DD, H, W] chunks into the full [B, 2C, D, H, W]."""
    full = np.empty((B, 2 * C, D, H, W), np.float32)
    for m in range(N_CORES):
        b, dh = divmod(m, 2)
        full[b, :, DD * dh : DD * dh + DD] = results[m]["out"]
    return full


def kernel(**inputs):
    nc = get_nc()
    in_maps = make_in_maps(inputs["left_feats"], inputs["right_feats"])
    res = run_bass_kernel_spmd(nc, in_maps, list(range(N_CORES))).results
    return assemble(res)


# revision 11
# speedup vs baseline: 4.0594x; 1.0387x over previous
"""Stereo cost-volume builder (nn_CostBuilder) as a Trainium2 Bass kernel.

Reference op: out[b, 0:C,  d, h, w] = left[b, c, h, w]   * (w >= d)
              out[b, C:2C, d, h, w] = right[b, c, h, w-d] * (w >= d)
with B=4, C=32, D=48, H=64, W=128 (f32). Output is [4, 64, 48, 64, 128].

Sharding across 8 cores: core m -> (b = m//2, d-half = m%2). Each core
produces out[b, :, d0:d0+24, :, :] (50.3 MB), i.e. both the left-masked and
right-shifted channels for 24 of the 48 disparities. The program is uniform
(true SPMD): the disparity offset d0 only changes per-core *data* (the mask
tensor and the host-side shift baked into the padded right features).

The op is write-bandwidth-bound (output is 48x the input), so the layout is
chosen to make output DMA descriptors fat: SBUF partition = (channel,
h-quarter) so each partition holds 16 h-rows, making every descriptor an
8 KB contiguous run on both the SBUF and DRAM side (v1 used 512 B runs and
was descriptor-rate-limited at ~197 GB/s/core).

Per d-chunk (tapered sizes 1,2,3,...,3,2,1 so the pipeline ramps fast and
drains short):
  - mask:  gpsimd iota (base=-d0k) + DVE is_ge against the per-core d0
           scalar -> 0/1 mask, no mask bytes read from HBM.
  - left:  one DVE tensor_mul [128, dc*16*128] = row * mask(d, w)
  - right: one ACT shifted copy [128, dc*16*128] from the zero-padded right
           rows (src AP steps: d=-1, h=+176, w=+1), realizing shift-by-d
           with zero fill.
  - one 1 MB DMA per (chunk, d', side): all 32 channels x 4 h-quarters =
    128 partitions -> 128 descriptors of 8 KB on one of the two HWDGE rings.
"""

import sys

if "/opt/trn_rl_repo" not in sys.path:
    sys.path.insert(0, "/opt/trn_rl_repo")

import numpy as np

import concourse.bacc as bacc
import concourse.bass as bass
import concourse.mybir as mybir
import concourse.tile as tile
from concourse.bass_utils import run_bass_kernel_spmd

B, C, H, W = 4, 32, 64, 128
D = 48          # MAX_DISP // 4
DD = D // 2     # disparities per core
N_CORES = 8
PAD = DD + DD + W  # 176 cols per padded right row
HP = 16         # h-rows per partition; partition = (c, h//HP), 32*4 = 128
NHQ = H // HP   # 4 h-quarters
CHUNKS = [1, 2, 3, 3, 3, 3, 3, 3, 2, 1]  # disparities per chunk (tapered
# head/tail so the first DMA starts early and the final drain is short)
assert sum(CHUNKS) == DD
FB = HP * W     # 2048: elements per (c, d, h-quarter) block = one 8KB descriptor

_NC_CACHE = {}


def _build_nc():
    nc = bacc.Bacc("TRN2", target_bir_lowering=False, debug=False)
    f32 = mybir.dt.float32

    lfeat = nc.dram_tensor("lfeat", [C, H, W], f32, kind="ExternalInput").ap()
    rpad = nc.dram_tensor("rpad", [C, H, PAD], f32, kind="ExternalInput").ap()
    dzero = nc.dram_tensor("dzero", [128, 1], f32, kind="ExternalInput").ap()
    out = nc.dram_tensor("out", [2 * C, DD, H, W], f32, kind="ExternalOutput").ap()

    c_str = DD * H * W  # 196608: channel stride in `out`

    with tile.TileContext(nc) as tc:
        with (
            tc.tile_pool(name="consts", bufs=1) as const_pool,
            tc.tile_pool(name="lst", bufs=3) as lst_pool,
            tc.tile_pool(name="rst", bufs=3) as rst_pool,
            tc.tile_pool(name="msk", bufs=2) as msk_pool,
        ):
            # whole-problem inputs, loaded once; one load per DMA path so they
            # run in parallel (sync/scalar = the two HWDGE rings, gpsimd = SWDGE)
            ltile = const_pool.tile([128, HP * W], f32, name="ltile")
            nc.sync.dma_start(ltile[:], lfeat[:])
            rtile = const_pool.tile([128, HP * PAD], f32, name="rtile")
            nc.scalar.dma_start(rtile[:], rpad[:])
            # per-core disparity offset, fed as a tiny [128,1] f32 input;
            # the rest of the mask is built on-device per chunk
            dztile = const_pool.tile([128, 1], f32, name="dztile")
            nc.gpsimd.dma_start(dztile[:], dzero)

            d0k = 0
            for k, dc in enumerate(CHUNKS):
                # mask for this chunk: mask[p, d'*W+w] = (w - (d0k+d') >= d0)
                itile = msk_pool.tile([128, dc * W], f32, name="itile")
                nc.gpsimd.iota(
                    itile[:],
                    [[-1, dc], [1, W]],
                    base=-d0k,
                    channel_multiplier=0,
                    allow_small_or_imprecise_dtypes=True,
                )
                mtile = msk_pool.tile([128, dc * W], f32, name="mtile")
                nc.vector.tensor_scalar(
                    out=mtile[:],
                    in0=itile[:],
                    scalar1=dztile[:],
                    scalar2=None,
                    op0=mybir.AluOpType.is_ge,
                )

                lstage = lst_pool.tile([128, dc * FB], f32, name="lstage", tag="lstage")
                rstage = rst_pool.tile([128, dc * FB], f32, name="rstage", tag="rstage")

                # left: lstage[p, d', hh, w] = ltile[p, hh, w] * mask[d0k+d', w]
                nc.vector.tensor_mul(
                    lstage[:].rearrange("p (d hh w) -> p d hh w", d=dc, hh=HP),
                    ltile[:]
                    .rearrange("p (hh w) -> p hh w", hh=HP)
                    .unsqueeze(1)
                    .to_broadcast((128, dc, HP, W)),
                    mtile[:]
                    .rearrange("p (d w) -> p d w", d=dc)
                    .unsqueeze(2)
                    .to_broadcast((128, dc, HP, W)),
                )

                # right: rstage[p, d', hh, w] = rtile[p, hh, DD + w - (d0k+d')]
                sR = rtile[:, (DD - d0k) : (DD - d0k) + 1]
                srcR = bass.AP(
                    sR.tensor,
                    sR.offset,
                    [[HP * PAD, 128], [-1, dc], [PAD, HP], [1, W]],
                )
                nc.scalar.copy(
                    rstage[:].rearrange("p (d hh w) -> p d hh w", d=dc, hh=HP),
                    srcR,
                )

                # DMAs out: one 1MB DMA per (chunk, d', side) covering all 32
                # channels x 4 h-quarters = 128 partitions -> 128 descriptors
                # of 8KB each, spread across all 16 SDMA ports.
                sL, sRs = lstage[:], rstage[:]
                for dp in range(dc):
                    srcLd = bass.AP(
                        sL.tensor,
                        sL.offset + dp * FB,
                        [[dc * FB, 128], [1, FB]],
                    )
                    dstLd = bass.AP(
                        out.tensor,
                        (d0k + dp) * H * W,
                        [[c_str, C], [FB, NHQ], [1, FB]],
                    )
                    nc.sync.dma_start(dstLd, srcLd)

                    srcRd = bass.AP(
                        sRs.tensor,
                        sRs.offset + dp * FB,
                        [[dc * FB, 128], [1, FB]],
                    )
                    dstRd = bass.AP(
                        out.tensor,
                        C * c_str + (d0k + dp) * H * W,
                        [[c_str, C], [FB, NHQ], [1, FB]],
                    )
                    nc.scalar.dma_start(dstRd, srcRd)
                d0k += dc

    nc.compile()
    return nc


def get_nc():
    if "nc" not in _NC_CACHE:
        _NC_CACHE["nc"] = _build_nc()
    return _NC_CACHE["nc"]


def make_in_maps(left, right):
    """Per-core input dicts for run_bass_kernel_spmd."""
    left = np.ascontiguousarray(left, dtype=np.float32)
    right = np.ascontiguousarray(right, dtype=np.float32)
    in_maps = []
    for m in range(N_CORES):
        b, dh = divmod(m, 2)
        d0 = DD * dh
        rpad = np.zeros((C, H, PAD), np.float32)
        rpad[:, :, DD + d0 : DD + d0 + W] = right[b]
        dzero = np.full((128, 1), d0, np.float32)
        in_maps.append(
            {"lfeat": np.ascontiguousarray(left[b]), "rpad": rpad, "dzero": dzero}
        )
    return in_maps


def assemble(results):
    """Gather per-core [2C, DD, H, W] chunks into the full [B, 2C, D, H, W]."""
    full = np.empty((B, 2 * C, D, H, W), np.float32)
    for m in range(N_CORES):
        b, dh = divmod(m, 2)
        full[b, :, DD * dh : DD * dh + DD] = results[m]["out"]
    return full


def kernel(**inputs):
    nc = get_nc()
    in_maps = make_in_maps(inputs["left_feats"], inputs["right_feats"])
    res = run_bass_kernel_spmd(nc, in_maps, list(range(N_CORES))).results
    return assemble(res)
